# revision 1
# baseline (speedup 1.0000x reference)
"""AttentionGate v2 on 8 trn2 NeuronCores.

Changes vs baseline:
  - Output bf16 (host casts to f32): halves output operand bytes.
  - Stores BOTH y_g and y_x (bf16) for both batch entries; the x-conv is never
    recomputed. Phase 2 is pointwise + tiny psi matmuls only.
  - One AllReduce per batch entry, issued right after its phase 1 so it
    overlaps the next phase's compute.
  - Tail channels (128:160) of g AND x are packed into one [128, S/2] tile per
    n (rows: g-even, g-odd, x-even, x-odd chunk quadrants) via zero-padded
    stationary weights, so all pointwise work runs 128 partitions wide and the
    psi tail contraction is a single matmul per chunk.
  - Weight-stationary matmul sweeps; [128,1024] psum groups with single big
    ACT copies; bn_stats everywhere; sqrt twice per call; tail stats folded
    across quadrants with one 0/1-matrix matmul.
"""

import sys

if "/opt/trn_rl_repo/concourse" not in sys.path:
    sys.path.insert(0, "/opt/trn_rl_repo/concourse")

import contextlib

import numpy as np
import ml_dtypes

import concourse.bass as bass
import concourse.bacc as bacc
import concourse.mybir as mybir
import concourse.tile as tile
from concourse.bass_utils import run_bass_kernel_spmd

F32 = mybir.dt.float32
BF16 = mybir.dt.bfloat16
F8 = mybir.dt.float8e4
BF = ml_dtypes.bfloat16
F8NP = ml_dtypes.float8_e4m3
AF = mybir.ActivationFunctionType
OP = mybir.AluOpType

N_CORES = 8
NB = 2          # batch
C = 320         # input channels
O = 160         # inter channels (128 main + 32 tail)
EPS = 1e-5
CH = 512        # pixels per chunk (one psum bank)
G1 = 1024       # phase-1 group / psi group / gating / ELU width


def build_kernel(S, n_cores=N_CORES):
    NCH = S // CH           # 512-chunks per n
    NG1 = S // G1           # 1024-groups per n
    assert S % G1 == 0 and (S // 2) % G1 == 0

    nc = bacc.Bacc("TRN2", target_bir_lowering=False, debug=False,
                   num_devices=n_cores)

    # g is only used by the conv; fp8 halves its per-call operand bytes.
    # InstanceNorm is scale-invariant so the host pre-scales Wg by 16 to
    # keep the fp8 weights in normal range -- the stats pass absorbs it.
    g_d = nc.dram_tensor("g", [NB, C, S], F8, kind="ExternalInput")
    x_d = nc.dram_tensor("x", [NB, C, S], BF16, kind="ExternalInput")
    # weight layout (host-built) [C, 384]:
    #   cols 0:128 = W.T mains
    #   cols 128+128v+rowoff(T)+32v : tails variant v, zero-padded elsewhere
    wg8_d = nc.dram_tensor("wg8", [128, 2, 384], F8, kind="ExternalInput")
    wg8c_d = nc.dram_tensor("wg8c", [64, 384], F8, kind="ExternalInput")
    wx_d = nc.dram_tensor("wx", [C, 384], BF16, kind="ExternalInput")
    wpm_d = nc.dram_tensor("wpm", [128, 1], BF16, kind="ExternalInput")
    # wpt2 col v: Wpsi tails at rows 32v:32v+32 AND rows 64+32v:96+32v
    wpt_d = nc.dram_tensor("wpt", [128, 2], BF16, kind="ExternalInput")
    cb_d = nc.dram_tensor("cb", [1, 1], F32, kind="ExternalInput")
    # tail fold+replicate matrix: fm[i,j] = 1 iff i%32==j%32 and i//64==j//64
    fm_d = nc.dram_tensor("fm", [128, 128], F32, kind="ExternalInput")
    out_d = nc.dram_tensor("out", [NB, C, S], BF16, kind="ExternalOutput")

    ar_in = [nc.dram_tensor(f"ar_in{n}", [128, 6], F32, kind="Internal")
             for n in range(NB)]
    ar_out = [nc.dram_tensor(f"ar_out{n}", [128, 6], F32, kind="Internal",
                             addr_space="Shared")
              for n in range(NB)]

    with tile.TileContext(nc) as tc, contextlib.ExitStack() as ctx:
        cpool = ctx.enter_context(tc.tile_pool(name="cpool", bufs=1))
        store = ctx.enter_context(tc.tile_pool(name="store", bufs=1))
        stats = ctx.enter_context(tc.tile_pool(name="stats", bufs=1))
        inp = ctx.enter_context(tc.tile_pool(name="inp", bufs=2))
        stage = ctx.enter_context(tc.tile_pool(name="stage", bufs=2))
        stage1 = ctx.enter_context(tc.tile_pool(name="stage1", bufs=1))
        stage2 = ctx.enter_context(tc.tile_pool(name="stage2", bufs=2))
        tiny = ctx.enter_context(tc.tile_pool(name="tiny", bufs=1))
        psm = ctx.enter_context(tc.tile_pool(name="psm", bufs=3, space="PSUM"))
        pst = ctx.enter_context(tc.tile_pool(name="pst", bufs=2, space="PSUM"))

        # ---- weights (emitted before phase1; first input DMAs are inside
        #      phase1 and share the sync queue) ----
        wgd = cpool.tile([128, 2, 384], F8, tag="wgd")
        wgc = cpool.tile([64, 384], F8, tag="wgc")
        wx0 = cpool.tile([128, 384], BF16, tag="wx0")
        wx1 = cpool.tile([128, 384], BF16, tag="wx1")
        wx2 = cpool.tile([64, 384], BF16, tag="wx2")
        nc.sync.dma_start(wgd[:], wg8_d.ap())
        nc.sync.dma_start(wgc[:], wg8c_d.ap())
        nc.sync.dma_start(wx0[:], wx_d[0:128, :])
        nc.sync.dma_start(wx1[:], wx_d[128:256, :])
        nc.sync.dma_start(wx2[:], wx_d[256:320, :])
        wpm = cpool.tile([128, 1], BF16, tag="wpm")
        wpt2 = cpool.tile([128, 2], BF16, tag="wpt2")
        nc.sync.dma_start(wpm[:], wpm_d.ap())
        nc.sync.dma_start(wpt2[:], wpt_d.ap())
        cbh = cpool.tile([1, 1], F32, tag="cbh")
        nc.sync.dma_start(cbh[:], cb_d.ap())
        fmt = cpool.tile([128, 128], F32, tag="fmt")
        nc.sync.dma_start(fmt[:], fm_d.ap())
        wsets = {"x": (wx0, wx1, wx2)}
        DR = mybir.MatmulPerfMode.DoubleRow

        # ---- persistent per-n storage ----
        ym, yt = {}, {}
        for n in range(NB):
            for T in ("g", "x"):
                ym[(T, n)] = store.tile([128, S], BF16,
                                        name=f"y{T}m{n}", tag=f"y{T}m{n}")
            yt[n] = store.tile([128, S // 2], BF16,
                               name=f"yt{n}", tag=f"yt{n}")

        # stats collect (shared across n: aggr(n) runs before phase1(n+1))
        sm = {T: stats.tile([128, NG1], F32, name=f"sm_{T}", tag=f"sm_{T}")
              for T in ("g", "x")}
        sq = {T: stats.tile([128, NG1], F32, name=f"sq_{T}", tag=f"sq_{T}")
              for T in ("g", "x")}
        smt = stats.tile([128, NG1], F32, name="smt", tag="smt")
        sqt = stats.tile([128, NG1], F32, name="sqt", tag="sqt")

        def phase1_gen(n):
            for u in range(NG1):       # 1024-px groups
                w0 = u * G1
                tin = {}
                for T in ("g", "x"):
                    src = g_d if T == "g" else x_d
                    dt_in = F8 if T == "g" else BF16
                    ia = inp.tile([128, 2, G1], dt_in, tag="ia", name=f"ia_{T}")
                    ib = inp.tile([64, G1], dt_in, tag="ib", name=f"ib_{T}")
                    nc.sync.dma_start(
                        ia[:],
                        src[n, 0:256, w0:w0 + G1]
                        .rearrange("(a p) c -> p a c", p=128))
                    nc.sync.dma_start(ib[:], src[n, 256:320, w0:w0 + G1])
                    tin[T] = (ia, ib)
                ptl = pst.tile([128, CH], F32, tag="ptl")
                # ---- g: fp8 DoubleRow contracts c-blocks 0+1 in one matmul
                ia, ib = tin["g"]
                pm = psm.tile([128, G1], F32, tag="pm", name="pm_g")
                for k in range(2):
                    kw = k * CH
                    nc.tensor.matmul(pm[:, kw:kw + CH], wgd[:, :, 0:128],
                                     ia[:, :, kw:kw + CH],
                                     start=True, stop=False, perf_mode=DR)
                    nc.tensor.matmul(pm[:, kw:kw + CH], wgc[:, 0:128],
                                     ib[:, kw:kw + CH],
                                     start=False, stop=True)
                for k in range(2):
                    kw = k * CH
                    c0 = 128 + 128 * k
                    nc.tensor.matmul(ptl[:], wgd[:, :, c0:c0 + 128],
                                     ia[:, :, kw:kw + CH],
                                     start=(k == 0), stop=False, perf_mode=DR)
                    nc.tensor.matmul(ptl[:], wgc[:, c0:c0 + 128],
                                     ib[:, kw:kw + CH],
                                     start=False, stop=False)
                emitted = {"g": pm}
                # ---- x: bf16 3-block sweep (unchanged)
                for T in ("x",):
                    wa, wb, wc = wsets[T]
                    ia, ib = tin[T]
                    pm = psm.tile([128, G1], F32, tag="pm", name=f"pm_{T}")
                    for b, wt in enumerate((wa, wb, wc)):
                        for k in range(2):
                            kw = k * CH
                            mv = (ib[:, kw:kw + CH] if b == 2
                                  else ia[:, b, kw:kw + CH])
                            nc.tensor.matmul(
                                pm[:, kw:kw + CH], wt[:, 0:128], mv,
                                start=(b == 0), stop=(b == 2))
                    for b, wt in enumerate((wa, wb, wc)):
                        for k in range(2):
                            kw = k * CH
                            mv = (ib[:, kw:kw + CH] if b == 2
                                  else ia[:, b, kw:kw + CH])
                            nc.tensor.matmul(
                                ptl[:], wt[:, 128 + 128 * k:256 + 128 * k], mv,
                                start=False,
                                stop=(b == 2 and k == 1))
                    emitted[T] = pm
                for T in ("g", "x"):
                    pmT = emitted[T]
                    # copy to store (ACT, with free running sum) + sumsq (DVE)
                    nc.scalar.activation(ym[(T, n)][:, w0:w0 + G1], pmT[:],
                                         AF.Copy,
                                         accum_out=sm[T][:, u:u + 1])
                    scr = stage.tile([128, G1], BF16, tag="e", name="scr")
                    nc.vector.scalar_tensor_tensor(
                        scr[:], ym[(T, n)][:, w0:w0 + G1], 1.0,
                        ym[(T, n)][:, w0:w0 + G1], OP.mult, OP.mult,
                        accum_out=sq[T][:, u:u + 1])
                nc.scalar.activation(yt[n][:, u * CH:(u + 1) * CH], ptl[:],
                                     AF.Copy, accum_out=smt[:, u:u + 1])
                scr2 = stage.tile([128, G1], BF16, tag="e", name="scr2")
                nc.vector.scalar_tensor_tensor(
                    scr2[:, 0:CH], yt[n][:, u * CH:(u + 1) * CH], 1.0,
                    yt[n][:, u * CH:(u + 1) * CH], OP.mult, OP.mult,
                    accum_out=sqt[:, u:u + 1])
                yield
            # aggregate + AllReduce ([128, 6]: raw sums/sumsqs)
            arst = tiny.tile([128, 6], F32, tag="arst")
            for col, cl in ((0, sm["g"]), (1, sq["g"]), (2, sm["x"]),
                            (3, sq["x"]), (4, smt), (5, sqt)):
                nc.vector.tensor_reduce(arst[:, col:col + 1], cl[:],
                                        mybir.AxisListType.X, OP.add)
            nc.sync.dma_start(ar_in[n].ap(), arst[:])
            nc.gpsimd.collective_compute(
                "AllReduce", OP.add,
                replica_groups=[list(range(n_cores))],
                ins=[ar_in[n].ap().opt()],
                outs=[ar_out[n].ap().opt()],
            )

        def consts(n):
            arb = tiny.tile([128, 6], F32, tag="arb")
            nc.sync.dma_start(arb[:], ar_out[n].ap())
            # fold tail quadrant pairs + replicate (one 0/1-matrix matmul)
            tfp = psm.tile([128, 2], F32, tag="pm", name=f"tfp{n}")
            nc.tensor.matmul(tfp[:], fmt[:], arb[:, 4:6], start=True, stop=True)
            tff = tiny.tile([128, 2], F32, tag="tff")
            nc.scalar.activation(tff[:], tfp[:], AF.Copy)
            # M3 / E3 cols = (g-main, x-main, tails); sums cover n_cores*S px
            inv = 1.0 / (n_cores * S)
            M3 = tiny.tile([128, 3], F32, tag="M3")
            E3 = tiny.tile([128, 3], F32, tag="E3")
            nc.vector.tensor_scalar(M3[:, 0:2], arb[:, 0:4:2], inv, None,
                                    OP.mult)
            nc.vector.tensor_scalar(M3[:, 2:3], tff[:, 0:1], inv, None, OP.mult)
            nc.vector.tensor_scalar(E3[:, 0:2], arb[:, 1:4:2], inv, EPS,
                                    OP.mult, OP.add)
            nc.vector.tensor_scalar(E3[:, 2:3], tff[:, 1:2], inv, EPS,
                                    OP.mult, OP.add)
            # var = E - M^2 (EPS already in E); r = sqrt(1/var); mr = mu*r
            v3 = tiny.tile([128, 3], F32, tag="v3")
            nc.vector.tensor_tensor(v3[:], M3[:], M3[:], OP.mult)
            nc.vector.tensor_tensor(v3[:], E3[:], v3[:], OP.subtract)
            rec3 = tiny.tile([128, 3], F32, tag="rec3")
            nc.vector.reciprocal(rec3[:], v3[:])
            r3 = tiny.tile([128, 3], F32, tag="r3")
            nc.scalar.activation(r3[:], rec3[:], AF.Sqrt)
            mr3 = tiny.tile([128, 3], F32, tag="mr3")
            nc.vector.tensor_tensor(mr3[:], M3[:], r3[:], OP.mult)
            nmr3 = tiny.tile([128, 3], F32, tag="nmr3")
            nc.vector.tensor_scalar(nmr3[:], mr3[:], -1.0, None, OP.mult)
            # psi bias: chalf = cb - 0.5*(wpm.(mr_gm+mr_xm+2) + wpt_v0.(mr_t+1))
            q = tiny.tile([128, 1], BF16, tag="q")
            qf = tiny.tile([128, 1], F32, tag="qf")
            nc.vector.tensor_tensor(qf[:], mr3[:, 0:1], mr3[:, 1:2], OP.add)
            nc.vector.tensor_scalar(q[:], qf[:], 1.0, 2.0, OP.mult, OP.add)
            qt = tiny.tile([128, 1], BF16, tag="qt")
            nc.vector.tensor_scalar(qt[:], mr3[:, 2:3], 1.0, 1.0, OP.mult,
                                    OP.add)
            dot = psm.tile([1, 1], F32, tag="pm", name=f"dot{n}")
            nc.tensor.matmul(dot[:], wpm[:], q[:], start=True, stop=False)
            nc.tensor.matmul(dot[:], wpt2[:, 0:1], qt[:], start=False, stop=True)
            chalf = tiny.tile([1, 1], F32, tag="chalf")
            nc.vector.tensor_scalar(chalf[:], dot[:], -0.5, cbh[:],
                                    OP.mult, OP.add)
            return {"r": r3, "mr": mr3, "nmr": nmr3, "chalf": chalf}

        def elu_inplace(dst, cols, cn, j, eng=None):
            """dst[:, cols] := max(r*y, mr) + min(exp(r*y + nmr), 1) in place."""
            eng = eng or nc.vector
            r = cn["r"][:, j:j + 1]
            mr = cn["mr"][:, j:j + 1]
            nmr = cn["nmr"][:, j:j + 1]
            e = stage.tile([128, G1], BF16, tag="e", name="e")
            nc.scalar.activation(e[:], dst[:, cols], AF.Exp,
                                 bias=nmr, scale=r)
            nc.vector.tensor_scalar(dst[:, cols], dst[:, cols], r, mr,
                                    OP.mult, OP.max)
            # dst += min(e, 1) fused: (e min 1) add dst
            eng.scalar_tensor_tensor(dst[:, cols], e[:], 1.0,
                                     dst[:, cols], OP.min, OP.add)

        def phase2_gen(n, cn):
            # tails ELU (one [128, S/2] tensor per n)
            for v in range(S // 2 // G1):
                elu_inplace(yt[n], slice(v * G1, (v + 1) * G1), cn, 2)
                yield
            chalf = cn["chalf"]
            for u in range(S // G1):
                hw0 = u * G1
                elu_inplace(ym[("g", n)], slice(hw0, hw0 + G1), cn, 0)
                elu_inplace(ym[("x", n)], slice(hw0, hw0 + G1), cn, 1)
                xa = inp.tile([128, 2, G1], BF16, tag="ia", name="xa")
                xb = inp.tile([64, G1], BF16, tag="ib", name="xb")
                nc.sync.dma_start(
                    xa[:], x_d[n, 0:256, hw0:hw0 + G1]
                    .rearrange("(a p) c -> p a c", p=128))
                nc.sync.dma_start(xb[:], x_d[n, 256:320, hw0:hw0 + G1])
                pp = psm.tile([1, G1], F32, tag="pm", name="pp")
                for k in range(2):
                    c = hw0 // CH + k
                    tcols = slice((c // 2) * CH, (c // 2 + 1) * CH)
                    ccols = slice(c * CH, (c + 1) * CH)
                    pk = pp[:, k * CH:(k + 1) * CH]
                    nc.tensor.matmul(pk, wpm[:], ym[("g", n)][:, ccols],
                                     start=True, stop=False)
                    nc.tensor.matmul(pk, wpm[:], ym[("x", n)][:, ccols],
                                     start=False, stop=False)
                    nc.tensor.matmul(pk, wpt2[:, (c % 2):(c % 2) + 1],
                                     yt[n][:, tcols],
                                     start=False, stop=True)
                pt_sb = stage.tile([1, G1], BF16, tag="pt", name="pt_sb")
                nc.scalar.activation(pt_sb[:], pp[:], AF.Tanh,
                                     bias=chalf[:], scale=0.5)
                nc.vector.tensor_scalar(pt_sb[:], pt_sb[:], 0.5, 0.5,
                                        OP.mult, OP.add)
                pb = stage.tile([128, G1], BF16, tag="pb", name="pb")
                nc.gpsimd.partition_broadcast(pb[:], pt_sb[:])
                om = stage2.tile([128, 2, G1], BF16, tag="om", name="om")
                o2t = stage2.tile([64, G1], BF16, tag="o2", name="o2t")
                nc.vector.tensor_tensor(om[:, 0, :], xa[:, 0, :], pb[:],
                                        OP.mult)
                nc.gpsimd.tensor_tensor(om[:, 1, :], xa[:, 1, :], pb[:],
                                        OP.mult)
                nc.vector.tensor_tensor(o2t[:], xb[:], pb[0:64, :],
                                        OP.mult)
                nc.sync.dma_start(
                    out_d[n, 0:256, hw0:hw0 + G1]
                    .rearrange("(a p) c -> p a c", p=128), om[:])
                nc.sync.dma_start(out_d[n, 256:320, hw0:hw0 + G1], o2t[:])
                yield

        # ---- emission schedule: overlap phase2(0) with the back half of
        #      phase1(1) (AR(0) has ~CG groups of cover before its first
        #      consumer, the consts(0) chain) ----
        NT2 = S // 2 // G1            # tails units in phase2
        for _ in phase1_gen(0):
            pass
        g1 = phase1_gen(1)
        CG = max(1, NG1 - max(1, NT2 // 2))
        for _ in range(CG):
            next(g1)
        cn0 = consts(0)
        p2 = phase2_gen(0, cn0)
        p2_alive = True
        for _ in range(NG1 - CG):
            next(g1)
            for _ in range(2):
                if p2_alive:
                    try:
                        next(p2)
                    except StopIteration:
                        p2_alive = False
        for _ in g1:
            pass
        while p2_alive:
            try:
                next(p2)
            except StopIteration:
                p2_alive = False
        cn1 = consts(1)
        for _ in phase2_gen(1, cn1):
            pass

    nc.compile()
    return nc


_CACHE = {}


def _get_nc(S, n_cores):
    key = (S, n_cores)
    if key not in _CACHE:
        _CACHE[key] = build_kernel(S, n_cores)
    return _CACHE[key]


def prep_weights(Wg, Wx, Wpsi, bpsi):
    """Host-side packing of the weight inputs (shared with test harness)."""
    def padded(WT, rowoff):
        # [C, 384]: cols 0:128 mains; variant v at cols 128+128v, with the
        # 32 tail columns placed at out-rows rowoff+32v.
        w = np.zeros((C, 384), np.float32)
        w[:, 0:128] = WT[:, 0:128]
        for v in range(2):
            o0 = 128 + 128 * v + rowoff + 32 * v
            w[:, o0:o0 + 32] = WT[:, 128:160]
        return w.astype(BF)

    wx = padded(np.ascontiguousarray(Wx.T), 64)
    wg8f = padded(np.ascontiguousarray(Wg.T) * 16.0, 0).astype(np.float32)
    wg8 = np.ascontiguousarray(
        wg8f[0:256].reshape(2, 128, 384).transpose(1, 0, 2)).astype(F8NP)
    wg8c = wg8f[256:320].astype(F8NP)
    wp = np.asarray(Wpsi).reshape(O)
    wpm = wp[0:128].reshape(128, 1).astype(BF)
    wpt = np.zeros((128, 2), np.float32)
    for v in range(2):
        wpt[32 * v:32 * v + 32, v] = wp[128:160]
        wpt[64 + 32 * v:96 + 32 * v, v] = wp[128:160]
    wpt = wpt.astype(BF)
    cb = np.array([[float(np.asarray(bpsi).reshape(-1)[0]) * 0.5]],
                  dtype=np.float32)
    ii = np.arange(128)
    fm = ((ii[:, None] % 32 == ii[None, :] % 32)
          & (ii[:, None] // 64 == ii[None, :] // 64)).astype(np.float32)
    return {"wg8": wg8, "wg8c": wg8c, "wx": wx, "wpm": wpm, "wpt": wpt,
            "cb": cb, "fm": fm}


def shard_inputs(g, x, Wg, Wx, Wpsi, bpsi, n_cores=N_CORES):
    """Build the per-core input maps (shared with test harness)."""
    n, c, d, h, w = g.shape
    dsh = d // n_cores
    S = dsh * h * w
    wts = prep_weights(Wg, Wx, Wpsi, bpsi)
    g5 = g.reshape(n, c, d, h * w)
    x5 = x.reshape(n, c, d, h * w)
    in_maps = []
    for cid in range(n_cores):
        dl, dh_ = cid * dsh, (cid + 1) * dsh
        gs = (np.ascontiguousarray(g5[:, :, dl:dh_]).astype(F8NP)
              .reshape(n, c, S))
        xs = np.ascontiguousarray(x5[:, :, dl:dh_]).astype(BF).reshape(n, c, S)
        in_maps.append({"g": gs, "x": xs, **wts})
    return in_maps, S


def kernel(g, x, Wg, bg, Wx, bx, Wpsi, bpsi):
    n, c, d, h, w = g.shape
    assert (n, c) == (NB, C)
    n_cores = N_CORES
    assert d % n_cores == 0
    in_maps, S = shard_inputs(g, x, Wg, Wx, Wpsi, bpsi, n_cores)
    nc = _get_nc(S, n_cores)
    res = run_bass_kernel_spmd(nc, in_maps, core_ids=list(range(n_cores)))
    dsh = d // n_cores
    out = np.empty((n, c, d, h * w), dtype=np.float32)
    for cid in range(n_cores):
        dl, dh_ = cid * dsh, (cid + 1) * dsh
        out[:, :, dl:dh_] = (res.results[cid]["out"].astype(np.float32)
                             .reshape(n, c, dsh, h * w))
    return out.reshape(n, c, d, h, w)

